# revision 5
# baseline (speedup 1.0000x reference)
"""Trainium2 Bass kernel for nn_BlankEmbedding (embedding gather + blank-run scan).

Math: the reference computes e = emb_table[x], then runs 8 iterations of
    pos = shift_right(pos); acc = shift_right(acc); out = out + acc; acc = out*pos
starting from pos = is_preblank (1 exactly at the position immediately before
the first blank of each contiguous run of blank tokens, ids 0..15).  Unrolling
the recurrence, out[i] = sum_{d=0..8} C[i,d] * e[i-d], where the banded
integer coefficients C depend only on x and satisfy
    C_0[i,d] = [d==0];  C_k[i,d] = C_{k-1}[i,d] + m[i-k] * C_{k-1}[i-1,d-1]
with m = is_preblank.  Rows with any C[i,d>0] != 0 are rare (~1/16 at the
reference's blank density), so the kernel is:

  per core (2048 of the 16384 rows, data-parallel over B*S):
    1. dma_gather the core's 2048 embedding rows from a deduplicated table
       (HBM->SBUF, 4 chunks of 512, ping-pong) and write each chunk to the
       output with a strided DMA.
    2. for affected rows (grouped <=128 per batch, sorted by band length):
       per-depth dma_gathers of the band rows e[i-d], DVE multiply-accumulate
       with per-partition scalar coefficients, then dma_scatter_add of the
       deltas onto the already-written output rows.

Host side only computes index lists / coefficients from x ([B,S] int ops) and
reassembles the 8 per-core outputs.
"""

import numpy as np

B, S, D = 4, 4096, 2048
N_CORES = 8
RPC = (B * S) // N_CORES          # rows per core = 2048
N_CHUNKS = 4
CHUNK = RPC // N_CHUNKS           # 512
GPP = CHUNK // 128                # rows per partition per chunk = 4
NBLANK_IDS = 16
N_ITER = 8
BAND = N_ITER + 1                 # out[i] depends on e[i-8..i]


def _cdiv(a, b):
    return (a + b - 1) // b


def _compute_coeffs(x):
    """C[b, s, d] for d=0..8 (float64 holds small ints exactly), per batch row."""
    b, s = x.shape
    blank = ((x >= 0) & (x < NBLANK_IDS)).astype(np.float64)
    shift_r = lambda t: np.concatenate([np.zeros_like(t[:, :1]), t[:, :-1]], axis=1)
    first = np.maximum(blank - shift_r(blank), 0.0)
    m = np.concatenate([first[:, 1:], np.zeros_like(first[:, :1])], axis=1)  # preblank
    C = np.zeros((b, s, BAND))
    C[:, :, 0] = 1.0
    for k in range(1, N_ITER + 1):
        m_k = np.zeros_like(m)
        m_k[:, k:] = m[:, :-k]                       # m[i-k]
        Cs = np.zeros_like(C)
        Cs[:, 1:, 1:] = C[:, :-1, :-1]               # C[i-1, d-1]
        C = C + m_k[:, :, None] * Cs
    return C


def _wrap16(vals, ncols):
    """Wrap a 1-D index list into the [128, ncols] int16 layout the SWDGE
    gather/scatter ucode expects: slot j at [j % 16, j // 16], and the 16-row
    block replicated across all eight 16-partition Q7 core groups."""
    blk = np.zeros((16, ncols), dtype=np.int16)
    v = np.asarray(vals, dtype=np.int16)
    for j in range(len(v)):
        blk[j % 16, j // 16] = v[j]
    return np.tile(blk, (8, 1))


def _prepare(x_np):
    """All host-side index/coefficient prep. Returns per-core arrays + meta."""
    uniq, inv = np.unique(x_np, return_inverse=True)
    ridx = inv.reshape(x_np.shape).astype(np.int64)   # x remapped to table rows
    NV = len(uniq)
    assert NV <= 32767, "int16 gather index overflow"

    C = _compute_coeffs(x_np)
    aff = (C[:, :, 1:] != 0).any(axis=2)              # [B,S]

    cores = []
    for c in range(N_CORES):
        b, h = c // 2, c % 2
        s0 = h * RPC
        # main gather indices, permuted so SBUF partition p holds rows p*GPP+g
        midx = np.zeros((128, N_CHUNKS * 32), dtype=np.int16)
        for ch in range(N_CHUNKS):
            slots = np.empty(CHUNK, dtype=np.int64)
            for j in range(CHUNK):
                l = (j % 128) * GPP + (j // 128) + ch * CHUNK
                slots[j] = ridx[b, s0 + l]
            midx[:, ch * 32:(ch + 1) * 32] = _wrap16(slots, 32)

        rows = np.nonzero(aff[b, s0:s0 + RPC])[0]     # local affected rows
        Cc = C[b, s0:s0 + RPC]                        # [RPC, 9] (local view)
        if len(rows):
            blen = np.array([np.nonzero(Cc[r, 1:])[0].max() + 1 for r in rows])
            order = np.argsort(-blen, kind="stable")
            rows = rows[order]
        cores.append(dict(b=b, s0=s0, rows=rows, Cc=Cc, midx=midx))

    G = max((_cdiv(len(co["rows"]), 128) for co in cores), default=0)
    G = max(G, 0)
    meta = dict(NV=NV, G=G, active=[])
    if G == 0:
        for co in cores:
            co.update(bidx=None, sidx=None, coef=None)
        return uniq, cores, meta

    # per (group, depth) gather length = max over cores, 16-aligned
    n_gd = np.zeros((G, N_ITER), dtype=np.int64)
    for co in cores:
        rows, Cc = co["rows"], co["Cc"]
        for g in range(G):
            rg = rows[g * 128:(g + 1) * 128]
            for d in range(1, N_ITER + 1):
                nz = np.nonzero(Cc[rg, d] != 0)[0]
                if len(nz):
                    n_gd[g, d - 1] = max(n_gd[g, d - 1], nz.max() + 1)
    n_gd = np.minimum(_cdiv(n_gd, 16) * 16, 128)
    meta["active"] = [
        [(d, int(n_gd[g, d - 1])) for d in range(1, N_ITER + 1) if n_gd[g, d - 1] > 0]
        for g in range(G)
    ]

    for co in cores:
        b, s0, rows, Cc = co["b"], co["s0"], co["rows"], co["Cc"]
        affset = set(rows.tolist())
        # pool of per-core safe (unaffected) local rows for scatter padding
        safe_pool = iter(r for r in range(RPC) if r not in affset)
        bidx = np.zeros((128, G * N_ITER * 8), dtype=np.int16)
        sidx = np.zeros((128, G * 8), dtype=np.int16)
        coef = np.zeros((128, G * N_ITER), dtype=np.float32)
        for g in range(G):
            rg = rows[g * 128:(g + 1) * 128]
            for d, n in meta["active"][g]:
                vals = np.zeros(n, dtype=np.int64)
                for r_i in range(n):
                    if r_i < len(rg) and Cc[rg[r_i], d] != 0:
                        lr = int(rg[r_i])
                        assert lr - d + s0 >= 0 or True
                        vals[r_i] = ridx[b, s0 + lr - d]
                        coef[r_i, g * N_ITER + d - 1] = Cc[rg[r_i], d]
                    else:
                        vals[r_i] = 0  # harmless pad read, coef stays 0
                blk = g * N_ITER + d - 1
                bidx[:, blk * 8: blk * 8 + _cdiv(n, 16)] = _wrap16(vals, _cdiv(n, 16))
            tgts = np.empty(128, dtype=np.int64)
            for sl in range(128):
                if sl < len(rg):
                    tgts[sl] = rg[sl]
                else:
                    tgts[sl] = next(safe_pool)  # distinct unaffected rows, +0.0
            sidx[:, g * 8:(g + 1) * 8] = _wrap16(tgts, 8)
        co.update(bidx=bidx, sidx=sidx, coef=coef)
    return uniq, cores, meta


def _build_program(NV, G, active):
    import concourse.bacc as bacc
    import concourse.mybir as mybir
    from concourse.library_config import mlp

    f32, i16 = mybir.dt.float32, mybir.dt.int16
    MULT, ADD = mybir.AluOpType.mult, mybir.AluOpType.add

    nc = bacc.Bacc("TRN2", target_bir_lowering=False, debug=False,
                   enable_asserts=False, num_devices=N_CORES)
    table = nc.dram_tensor("table", [NV, D], f32, kind="ExternalInput")
    midx_d = nc.dram_tensor("midx", [128, N_CHUNKS * 32], i16, kind="ExternalInput")
    out_d = nc.dram_tensor("out", [RPC, D], f32, kind="ExternalOutput")
    if G:
        bidx_d = nc.dram_tensor("bidx", [128, G * N_ITER * 8], i16, kind="ExternalInput")
        sidx_d = nc.dram_tensor("sidx", [128, G * 8], i16, kind="ExternalInput")
        coef_d = nc.dram_tensor("coef", [128, G * N_ITER], f32, kind="ExternalInput")

    from contextlib import ExitStack
    with ExitStack() as st:
        mbuf = [st.enter_context(nc.sbuf_tensor(f"mbuf{i}", [128, GPP, D], f32))
                for i in range(2)]
        midx_s = st.enter_context(nc.sbuf_tensor("midx_s", [128, N_CHUNKS * 32], i16))
        idx_sem = st.enter_context(nc.semaphore("idx_sem"))
        g_sem = st.enter_context(nc.semaphore("g_sem"))
        w_sem = st.enter_context(nc.semaphore("w_sem"))
        if G:
            band = st.enter_context(nc.sbuf_tensor("band", [128, N_ITER, D], f32))
            scratch = st.enter_context(nc.sbuf_tensor("scratch", [128, 1, D], f32))
            finals = [st.enter_context(nc.sbuf_tensor(f"final{g}", [128, 1, D], f32))
                      for g in range(G)]
            bidx_s = st.enter_context(nc.sbuf_tensor("bidx_s", [128, G * N_ITER * 8], i16))
            sidx_s = st.enter_context(nc.sbuf_tensor("sidx_s", [128, G * 8], i16))
            coef_s = st.enter_context(nc.sbuf_tensor("coef_s", [128, G * N_ITER], f32))
            ms_sem = st.enter_context(nc.semaphore("ms_sem"))
            b_sem = st.enter_context(nc.semaphore("b_sem"))
            d_sem = st.enter_context(nc.semaphore("d_sem"))
            s_sem = st.enter_context(nc.semaphore("s_sem"))
        block = st.enter_context(nc.Block())
        n_idx_dmas = 1 + (3 if G else 0)

        @block.sync
        def _(sync):
            sync.dma_start(midx_s[:, :], midx_d[:, :]).then_inc(idx_sem, 16)
            if G:
                sync.dma_start(bidx_s[:, :], bidx_d[:, :]).then_inc(idx_sem, 16)
                sync.dma_start(sidx_s[:, :], sidx_d[:, :]).then_inc(idx_sem, 16)
                sync.dma_start(coef_s[:, :], coef_d[:, :]).then_inc(idx_sem, 16)
            for ch in range(N_CHUNKS):
                sync.wait_ge(g_sem, 16 * (ch + 1))
                dst = out_d[ch * CHUNK:(ch + 1) * CHUNK, :].rearrange(
                    "(p g) e -> p g e", g=GPP)
                sync.dma_start(dst, mbuf[ch % 2][:, :, :]).then_inc(w_sem, 16)

        @block.gpsimd
        def _(gp):
            gp.load_library(mlp)
            gp.wait_ge(idx_sem, 16 * n_idx_dmas)
            for ch in range(N_CHUNKS):
                if ch >= 2:
                    gp.wait_ge(w_sem, 16 * (ch - 1))
                gp.dma_gather(mbuf[ch % 2][:, :, :], table[:, :],
                              midx_s[:, ch * 32:(ch + 1) * 32],
                              CHUNK, CHUNK, D).then_inc(g_sem, 16)
            if G:
                gp.wait_ge(idx_sem, 16 * n_idx_dmas)
                gp.wait_ge(ms_sem, 1)
                for g in range(G):
                    if g >= 1:
                        gp.wait_ge(d_sem, g)   # DVE done reading band (WAR)
                    for d, n in active[g]:
                        blk = g * N_ITER + d - 1
                        gp.dma_gather(band[:, d - 1:d, :], table[:, :],
                                      bidx_s[:, blk * 8: blk * 8 + _cdiv(n, 16)],
                                      n, n, D).then_inc(b_sem, 16)
                gp.wait_ge(w_sem, 16 * N_CHUNKS)   # all base rows written
                for g in range(G):
                    gp.wait_ge(d_sem, g + 1)       # delta ready
                    gp.dma_scatter_add(out_d[:, :], finals[g][:, :, :],
                                       sidx_s[:, g * 8:(g + 1) * 8],
                                       128, 128, D).then_inc(s_sem, 16)
                gp.wait_ge(s_sem, 16 * G)

        @block.vector
        def _(v):
            if not G:
                return
            v.memset(band[:, :, :], 0.0).then_inc(ms_sem, 1)
            v.wait_ge(idx_sem, 16 * n_idx_dmas)
            nb = 0
            for g in range(G):
                nb += len(active[g])
                v.wait_ge(b_sem, 16 * nb)
                L = len(active[g])
                prev = None
                for i, (d, n) in enumerate(active[g]):
                    scl = coef_s[:, g * N_ITER + d - 1: g * N_ITER + d]
                    src = band[:, d - 1, :]
                    dst = finals[g] if (L - 1 - i) % 2 == 0 else scratch
                    if prev is None:
                        ins = v.tensor_scalar_mul(dst[:, 0, :], src, scl)
                    else:
                        ins = v.scalar_tensor_tensor(dst[:, 0, :], src, scl,
                                                     prev[:, 0, :], MULT, ADD)
                    prev = dst
                ins.then_inc(d_sem, 1)

    nc.compile()
    return nc


_CACHE = {}
_LAST_RESULT = None


def kernel(x, emb_table):
    global _LAST_RESULT
    from concourse.bass_utils import run_bass_kernel_spmd

    x_np = np.asarray(x)
    emb_np = np.asarray(emb_table, dtype=np.float32)
    uniq, cores, meta = _prepare(x_np)
    table_sl = np.ascontiguousarray(emb_np[uniq])

    key = (meta["NV"], meta["G"], tuple(tuple(a) for a in meta["active"]))
    if key not in _CACHE:
        _CACHE[key] = _build_program(meta["NV"], meta["G"], meta["active"])
    nc = _CACHE[key]

    in_maps = []
    for co in cores:
        m = {"table": table_sl, "midx": co["midx"]}
        if meta["G"]:
            m.update(bidx=co["bidx"], sidx=co["sidx"], coef=co["coef"])
        in_maps.append(m)

    res = run_bass_kernel_spmd(nc, in_maps, core_ids=list(range(N_CORES)))
    _LAST_RESULT = res
    full = np.empty((B, S, D), dtype=np.float32)
    for c in range(N_CORES):
        b, h = c // 2, c % 2
        full[b, h * RPC:(h + 1) * RPC, :] = res.results[c]["out"]
    return full


# revision 6
# speedup vs baseline: 1.1040x; 1.1040x over previous
"""Trainium2 Bass kernel for nn_BlankEmbedding (embedding gather + blank-run scan).

Math: the reference computes e = emb_table[x], then runs 8 iterations of
    pos = shift_right(pos); acc = shift_right(acc); out = out + acc; acc = out*pos
starting from pos = is_preblank (1 exactly at the position immediately before
the first blank of each contiguous run of blank tokens, ids 0..15).  Unrolling
the recurrence, out[i] = sum_{d=0..8} C[i,d] * e[i-d], where the banded
integer coefficients C depend only on x and satisfy
    C_0[i,d] = [d==0];  C_k[i,d] = C_{k-1}[i,d] + m[i-k] * C_{k-1}[i-1,d-1]
with m = is_preblank.  Rows with any C[i,d>0] != 0 are rare (~1/16 at the
reference's blank density), so the kernel is:

  per core (2048 of the 16384 rows, data-parallel over B*S):
    1. dma_gather the core's 2048 embedding rows from a deduplicated table
       (HBM->SBUF, 4 chunks of 512, ping-pong) and write each chunk to the
       output with a strided DMA.
    2. for affected rows (grouped <=128 per batch, sorted by band length):
       per-depth dma_gathers of the band rows e[i-d], DVE multiply-accumulate
       with per-partition scalar coefficients, then dma_scatter_add of the
       deltas onto the already-written output rows.

Host side only computes index lists / coefficients from x ([B,S] int ops) and
reassembles the 8 per-core outputs.
"""

import numpy as np

B, S, D = 4, 4096, 2048
N_CORES = 8
RPC = (B * S) // N_CORES          # rows per core = 2048
N_CHUNKS = 4
CHUNK = RPC // N_CHUNKS           # 512
GPP = CHUNK // 128                # rows per partition per chunk = 4
NBLANK_IDS = 16
N_ITER = 8
BAND = N_ITER + 1                 # out[i] depends on e[i-8..i]


def _cdiv(a, b):
    return (a + b - 1) // b


def _compute_coeffs(x):
    """C[b, s, d] for d=0..8 (float64 holds small ints exactly), per batch row."""
    b, s = x.shape
    blank = ((x >= 0) & (x < NBLANK_IDS)).astype(np.float64)
    shift_r = lambda t: np.concatenate([np.zeros_like(t[:, :1]), t[:, :-1]], axis=1)
    first = np.maximum(blank - shift_r(blank), 0.0)
    m = np.concatenate([first[:, 1:], np.zeros_like(first[:, :1])], axis=1)  # preblank
    C = np.zeros((b, s, BAND))
    C[:, :, 0] = 1.0
    for k in range(1, N_ITER + 1):
        m_k = np.zeros_like(m)
        m_k[:, k:] = m[:, :-k]                       # m[i-k]
        Cs = np.zeros_like(C)
        Cs[:, 1:, 1:] = C[:, :-1, :-1]               # C[i-1, d-1]
        C = C + m_k[:, :, None] * Cs
    return C


def _wrap16(vals, ncols):
    """Wrap a 1-D index list into the [128, ncols] int16 layout the SWDGE
    gather/scatter ucode expects: slot j at [j % 16, j // 16], and the 16-row
    block replicated across all eight 16-partition Q7 core groups."""
    blk = np.zeros((16, ncols), dtype=np.int16)
    v = np.asarray(vals, dtype=np.int16)
    for j in range(len(v)):
        blk[j % 16, j // 16] = v[j]
    return np.tile(blk, (8, 1))


def _prepare(x_np):
    """All host-side index/coefficient prep. Returns per-core arrays + meta."""
    uniq, inv = np.unique(x_np, return_inverse=True)
    ridx = inv.reshape(x_np.shape).astype(np.int64)   # x remapped to table rows
    NV = len(uniq)
    assert NV <= 32767, "int16 gather index overflow"

    C = _compute_coeffs(x_np)
    aff = (C[:, :, 1:] != 0).any(axis=2)              # [B,S]

    cores = []
    for c in range(N_CORES):
        b, h = c // 2, c % 2
        s0 = h * RPC
        # main gather indices, permuted so SBUF partition p holds rows p*GPP+g
        midx = np.zeros((128, N_CHUNKS * 32), dtype=np.int16)
        for ch in range(N_CHUNKS):
            slots = np.empty(CHUNK, dtype=np.int64)
            for j in range(CHUNK):
                l = (j % 128) * GPP + (j // 128) + ch * CHUNK
                slots[j] = ridx[b, s0 + l]
            midx[:, ch * 32:(ch + 1) * 32] = _wrap16(slots, 32)

        rows = np.nonzero(aff[b, s0:s0 + RPC])[0]     # local affected rows
        Cc = C[b, s0:s0 + RPC]                        # [RPC, 9] (local view)
        if len(rows):
            blen = np.array([np.nonzero(Cc[r, 1:])[0].max() + 1 for r in rows])
            order = np.argsort(-blen, kind="stable")
            rows = rows[order]
        cores.append(dict(b=b, s0=s0, rows=rows, Cc=Cc, midx=midx))

    G = max((_cdiv(len(co["rows"]), 128) for co in cores), default=0)
    G = max(G, 0)
    meta = dict(NV=NV, G=G, active=[])
    if G == 0:
        for co in cores:
            co.update(bidx=None, sidx=None, coef=None)
        return uniq, cores, meta

    # per (group, depth) gather length = max over cores, 16-aligned
    n_gd = np.zeros((G, N_ITER), dtype=np.int64)
    for co in cores:
        rows, Cc = co["rows"], co["Cc"]
        for g in range(G):
            rg = rows[g * 128:(g + 1) * 128]
            for d in range(1, N_ITER + 1):
                nz = np.nonzero(Cc[rg, d] != 0)[0]
                if len(nz):
                    n_gd[g, d - 1] = max(n_gd[g, d - 1], nz.max() + 1)
    n_gd = np.minimum(_cdiv(n_gd, 16) * 16, 128)
    meta["active"] = [
        [(d, int(n_gd[g, d - 1])) for d in range(1, N_ITER + 1) if n_gd[g, d - 1] > 0]
        for g in range(G)
    ]

    for co in cores:
        b, s0, rows, Cc = co["b"], co["s0"], co["rows"], co["Cc"]
        affset = set(rows.tolist())
        # pool of per-core safe (unaffected) local rows for scatter padding
        safe_pool = iter(r for r in range(RPC) if r not in affset)
        bidx = np.zeros((128, G * N_ITER * 8), dtype=np.int16)
        sidx = np.zeros((128, G * 8), dtype=np.int16)
        coef = np.zeros((128, G * N_ITER), dtype=np.float32)
        for g in range(G):
            rg = rows[g * 128:(g + 1) * 128]
            for d, n in meta["active"][g]:
                vals = np.zeros(n, dtype=np.int64)
                for r_i in range(n):
                    if r_i < len(rg) and Cc[rg[r_i], d] != 0:
                        lr = int(rg[r_i])
                        assert lr - d + s0 >= 0 or True
                        vals[r_i] = ridx[b, s0 + lr - d]
                        coef[r_i, g * N_ITER + d - 1] = Cc[rg[r_i], d]
                    else:
                        vals[r_i] = 0  # harmless pad read, coef stays 0
                blk = g * N_ITER + d - 1
                bidx[:, blk * 8: blk * 8 + _cdiv(n, 16)] = _wrap16(vals, _cdiv(n, 16))
            tgts = np.empty(128, dtype=np.int64)
            for sl in range(128):
                if sl < len(rg):
                    tgts[sl] = rg[sl]
                else:
                    tgts[sl] = next(safe_pool)  # distinct unaffected rows, +0.0
            sidx[:, g * 8:(g + 1) * 8] = _wrap16(tgts, 8)
        co.update(bidx=bidx, sidx=sidx, coef=coef)
    return uniq, cores, meta


def _build_program(NV, G, active):
    import concourse.bacc as bacc
    import concourse.mybir as mybir
    from concourse.library_config import mlp

    f32, i16 = mybir.dt.float32, mybir.dt.int16
    MULT, ADD = mybir.AluOpType.mult, mybir.AluOpType.add

    nc = bacc.Bacc("TRN2", target_bir_lowering=False, debug=False,
                   enable_asserts=False, num_devices=N_CORES)
    table = nc.dram_tensor("table", [NV, D], f32, kind="ExternalInput")
    midx_d = nc.dram_tensor("midx", [128, N_CHUNKS * 32], i16, kind="ExternalInput")
    out_d = nc.dram_tensor("out", [RPC, D], f32, kind="ExternalOutput")
    if G:
        bidx_d = nc.dram_tensor("bidx", [128, G * N_ITER * 8], i16, kind="ExternalInput")
        sidx_d = nc.dram_tensor("sidx", [128, G * 8], i16, kind="ExternalInput")
        coef_d = nc.dram_tensor("coef", [128, G * N_ITER], f32, kind="ExternalInput")

    from contextlib import ExitStack
    with ExitStack() as st:
        mbuf = [st.enter_context(nc.sbuf_tensor(f"mbuf{i}", [128, GPP, D], f32))
                for i in range(2)]
        midx_s = st.enter_context(nc.sbuf_tensor("midx_s", [128, N_CHUNKS * 32], i16))
        idx_sem = st.enter_context(nc.semaphore("idx_sem"))
        g_sem = st.enter_context(nc.semaphore("g_sem"))
        w_sem = st.enter_context(nc.semaphore("w_sem"))
        if G:
            # per-group band tiles sized to that group's active depth list
            bands = [st.enter_context(
                nc.sbuf_tensor(f"band{g}", [128, max(len(active[g]), 1), D], f32))
                for g in range(G)]
            scratch = st.enter_context(nc.sbuf_tensor("scratch", [128, 1, D], f32))
            finals = [st.enter_context(nc.sbuf_tensor(f"final{g}", [128, 1, D], f32))
                      for g in range(G)]
            bidx_s = st.enter_context(nc.sbuf_tensor("bidx_s", [128, G * N_ITER * 8], i16))
            sidx_s = st.enter_context(nc.sbuf_tensor("sidx_s", [128, G * 8], i16))
            coef_s = st.enter_context(nc.sbuf_tensor("coef_s", [128, G * N_ITER], f32))
            ms_sem = st.enter_context(nc.semaphore("ms_sem"))
            b_sem = st.enter_context(nc.semaphore("b_sem"))
            d_sem = st.enter_context(nc.semaphore("d_sem"))
            s_sem = st.enter_context(nc.semaphore("s_sem"))
        block = st.enter_context(nc.Block())
        n_idx_dmas = 1 + (3 if G else 0)

        @block.sync
        def _(sync):
            sync.dma_start(midx_s[:, :], midx_d[:, :]).then_inc(idx_sem, 16)
            if G:
                sync.dma_start(bidx_s[:, :], bidx_d[:, :]).then_inc(idx_sem, 16)
                sync.dma_start(sidx_s[:, :], sidx_d[:, :]).then_inc(idx_sem, 16)
                sync.dma_start(coef_s[:, :], coef_d[:, :]).then_inc(idx_sem, 16)
            for ch in range(N_CHUNKS):
                sync.wait_ge(g_sem, 16 * (ch + 1))
                dst = out_d[ch * CHUNK:(ch + 1) * CHUNK, :].rearrange(
                    "(p g) e -> p g e", g=GPP)
                sync.dma_start(dst, mbuf[ch % 2][:, :, :]).then_inc(w_sem, 16)

        @block.gpsimd
        def _(gp):
            gp.load_library(mlp)
            gp.wait_ge(idx_sem, 16 * n_idx_dmas)

            def main_gather(ch):
                gp.dma_gather(mbuf[ch % 2][:, :, :], table[:, :],
                              midx_s[:, ch * 32:(ch + 1) * 32],
                              CHUNK, CHUNK, D,
                              single_packet=False).then_inc(g_sem, 16)

            main_gather(0)
            main_gather(1)
            if G:
                # band gathers early: their data only depends on the table,
                # so deltas can be computed while the main pipeline runs
                gp.wait_ge(ms_sem, 1)
                for g in range(G):
                    for j, (d, n) in enumerate(active[g]):
                        blk = g * N_ITER + d - 1
                        gp.dma_gather(bands[g][:, j:j + 1, :], table[:, :],
                                      bidx_s[:, blk * 8: blk * 8 + _cdiv(n, 16)],
                                      n, n, D,
                                      single_packet=False).then_inc(b_sem, 16)
            for ch in range(2, N_CHUNKS):
                gp.wait_ge(w_sem, 16 * (ch - 1))
                main_gather(ch)
            if G:
                gp.wait_ge(w_sem, 16 * N_CHUNKS)   # all base rows written
                for g in range(G):
                    gp.wait_ge(d_sem, g + 1)       # delta ready
                    gp.dma_scatter_add(out_d[:, :], finals[g][:, :, :],
                                       sidx_s[:, g * 8:(g + 1) * 8],
                                       128, 128, D,
                                       single_packet=False).then_inc(s_sem, 16)
                gp.wait_ge(s_sem, 16 * G)

        @block.vector
        def _(v):
            if not G:
                return
            for g in range(G):
                ins = v.memset(bands[g][:, :, :], 0.0)
            ins.then_inc(ms_sem, 1)
            v.wait_ge(idx_sem, 16 * n_idx_dmas)
            nb = 0
            for g in range(G):
                nb += len(active[g])
                v.wait_ge(b_sem, 16 * nb)
                L = len(active[g])
                prev = None
                for j, (d, n) in enumerate(active[g]):
                    scl = coef_s[:, g * N_ITER + d - 1: g * N_ITER + d]
                    src = bands[g][:, j, :]
                    dst = finals[g] if (L - 1 - j) % 2 == 0 else scratch
                    if prev is None:
                        ins = v.tensor_scalar_mul(dst[:, 0, :], src, scl)
                    else:
                        ins = v.scalar_tensor_tensor(dst[:, 0, :], src, scl,
                                                     prev[:, 0, :], MULT, ADD)
                    prev = dst
                ins.then_inc(d_sem, 1)

    nc.compile()
    return nc


_CACHE = {}
_LAST_RESULT = None


def kernel(x, emb_table):
    global _LAST_RESULT
    from concourse.bass_utils import run_bass_kernel_spmd

    x_np = np.asarray(x)
    emb_np = np.asarray(emb_table, dtype=np.float32)
    uniq, cores, meta = _prepare(x_np)
    table_sl = np.ascontiguousarray(emb_np[uniq])

    key = (meta["NV"], meta["G"], tuple(tuple(a) for a in meta["active"]))
    if key not in _CACHE:
        _CACHE[key] = _build_program(meta["NV"], meta["G"], meta["active"])
    nc = _CACHE[key]

    in_maps = []
    for co in cores:
        m = {"table": table_sl, "midx": co["midx"]}
        if meta["G"]:
            m.update(bidx=co["bidx"], sidx=co["sidx"], coef=co["coef"])
        in_maps.append(m)

    res = run_bass_kernel_spmd(nc, in_maps, core_ids=list(range(N_CORES)))
    _LAST_RESULT = res
    full = np.empty((B, S, D), dtype=np.float32)
    for c in range(N_CORES):
        b, h = c // 2, c % 2
        full[b, h * RPC:(h + 1) * RPC, :] = res.results[c]["out"]
    return full
